# revision 14
# baseline (speedup 1.0000x reference)
"""ConvLSTM3D encoder for 8 trn2 NeuronCores — halo-exchange version.

Sharding: core c handles batch c//4, z-quarter q = c%4 (output planes
[8q, 8q+8)).  Unlike the collective-free baseline (which replicated a
shrinking 26->8 plane window, 144 plane-steps/core), each core computes
exactly its 8 planes every step (72 plane-steps/core) and exchanges the
two boundary h planes with its z-neighbors after every step via a
4-core AllGather (groups {0..3}, {4..7}).

SPMD uniformity: the gathered buffer holds all 4 ranks' (bottom, top)
planes; each core extracts ITS two halo planes with one-hot selection
matmuls whose [128,32] selector matrices are per-core input data (edge
quarters get all-zero selectors -> zero halo = 'same' conv padding).

h-stack: double-buffered [124, 8 slots, 34, 34] (bf16).  Partitions
0..95 hold 3 z-shift groups x 32 ch: group g at slot s holds plane
8q+s-1+g of the previous step's h, zero-padded borders.  Partitions
96..123 carry the per-step x-im2col taps (27) + ones row; they ride
pass 0 of the 9 (dy,dx) matmul passes, which also adds Wx-conv + bias.

Per plane: 9 matmul passes x 2 half-planes -> PSUM -> sigmoid/tanh ->
gate-major 'gates' -> 4 partition-crossing DMAs -> plane-major 'gt' ->
LSTM pointwise on DVE -> h_t copied into the write-stack (<=3 shift
slots).  c state never moves (fixed plane ownership).

Step slice order {0,7} -> {1..4} -> {5,6} puts the boundary planes
first so the AllGather overlaps the interior compute; the selection
matmuls are emitted after the last interior matmul so the in-order
tensor queue never stalls on the collective.
"""

import os
import sys
from contextlib import ExitStack

import numpy as np

for _p in ("/opt/trn_rl_repo", "/root/.axon_site/_ro/trn_rl_repo"):
    if os.path.isdir(_p) and _p not in sys.path:
        sys.path.insert(0, _p)

import concourse.bass as bass
import concourse.bacc as bacc
import concourse.mybir as mybir
from concourse import tile
from concourse.bass_utils import run_bass_kernel_spmd

F32 = mybir.dt.float32
MM_DT = mybir.dt.bfloat16
GT_DT = mybir.dt.bfloat16

T = 10             # total steps; step 0 on host, device runs t=1..9
CH = 32            # hidden channels
PLW = 34           # padded plane width
PL = PLW * PLW     # padded plane elements (1156)
NROW = 124         # contraction rows: 96 h + 27 x-taps + ones
DELTAS = [(dy, dx) for dy in range(3) for dx in range(3)]
# slices in issue order: interior B first (no halo dep -> the previous
# step's exchange gets B's whole duration as slack), then boundary A
# (computes + ships the exchange planes), then interior C.
SLICES = [("B", [1, 2, 3, 4], 1), ("A", [0, 7], 0), ("C", [5, 6], 2)]
GROUPS = [[0, 1, 2, 3], [4, 5, 6, 7]]

_prog_cache = {}


def _build_program():
    if "nc" in _prog_cache:
        return _prog_cache["nc"]

    nc = bacc.Bacc(num_devices=8)

    stk0_d = nc.dram_tensor("stk0", [NROW, 8 * PL], MM_DT, kind="ExternalInput")
    zer_d = nc.dram_tensor("zer", [96, 8 * PL], MM_DT, kind="ExternalInput")
    xim_d = nc.dram_tensor("xim", [8, 28, 8, PL], MM_DT, kind="ExternalInput")
    whl_d = nc.dram_tensor("whl", [9, NROW, 128], MM_DT, kind="ExternalInput")
    c0_d = nc.dram_tensor("c0", [128, 3 * 1024], F32, kind="ExternalInput")
    slo_d = nc.dram_tensor("selLo", [128, 32], MM_DT, kind="ExternalInput")
    shi_d = nc.dram_tensor("selHi", [128, 32], MM_DT, kind="ExternalInput")
    hout_d = nc.dram_tensor("hout", [CH, 8, 32, 32], MM_DT,
                            kind="ExternalOutput")

    with ExitStack() as ctx:
        tc = ctx.enter_context(tile.TileContext(nc))
        pers = ctx.enter_context(tc.tile_pool(name="pers", bufs=1))
        psum = ctx.enter_context(tc.tile_pool(name="psum", bufs=3, space="PSUM"))
        work = ctx.enter_context(tc.tile_pool(name="work", bufs=2))
        dram = ctx.enter_context(tc.tile_pool(name="dram", bufs=2, space="DRAM"))

        stkA = pers.tile([128, 8 * PL], MM_DT, tag="stkA", name="stkA")
        stkB = pers.tile([128, 8 * PL], MM_DT, tag="stkB", name="stkB")
        wh_sb = pers.tile([128, 9 * 128], MM_DT, tag="wh", name="wh_sb")
        cst = pers.tile([128, 3 * 1024], F32, tag="cst", name="cst")
        slo = pers.tile([128, 32], MM_DT, tag="slo", name="slo")
        shi = pers.tile([128, 32], MM_DT, tag="shi", name="shi")

        ld_engines = [nc.sync, nc.scalar, nc.gpsimd]
        for d in range(9):
            ld_engines[d % 3].dma_start(
                out=wh_sb[0:NROW, 128 * d:128 * (d + 1)], in_=whl_d[d])
        nc.sync.dma_start(out=slo[:, :], in_=slo_d[:, :])
        nc.scalar.dma_start(out=shi[:, :], in_=shi_d[:, :])

        # initial stack A (h_0 3-group + xim[t=1] + ones), B zeroed h-part
        quarter = 8 * PL // 4
        for qq in range(4):
            lo, hi = qq * quarter, (qq + 1) * quarter
            ld_engines[qq % 3].dma_start(out=stkA[0:NROW, lo:hi],
                                         in_=stk0_d[:, lo:hi])
            ld_engines[(qq + 1) % 3].dma_start(out=stkB[0:96, lo:hi],
                                               in_=zer_d[:, lo:hi])
        for qq in range(3):
            ld_engines[qq].dma_start(out=cst[:, 1024 * qq:1024 * (qq + 1)],
                                     in_=c0_d[:, 1024 * qq:1024 * (qq + 1)])

        stacks = [stkA, stkB]
        dma_engines = [nc.sync, nc.scalar]
        dma_i = 0

        for t in range(1, T):
            R = stacks[(t + 1) % 2]          # read stack (A at t=1)
            W = stacks[t % 2]                # write stack
            Rv = R[:, :].rearrange("p (s y x) -> p s y x", s=8, y=PLW, x=PLW)
            Wv = W[:, :].rearrange("p (s y x) -> p s y x", s=8, y=PLW, x=PLW)
            Wv2 = W[:, :].rearrange("p (s f) -> p s f", s=8, f=PL)

            # prefetch next step's x-im2col into the write stack's x rows
            # (WAR on step t-1's matmuls resolves right at step start)
            if t < T - 1:
                nc.gpsimd.dma_start(out=Wv2[96:124, :, :], in_=xim_d[t - 1])

            for nm, js, blk in SLICES:
                npl = len(js)
                PP = 32 * npl
                gates = work.tile([128, 4096], GT_DT, tag="gates", name="gates")
                gt = work.tile([128, 4096], GT_DT, tag="gt", name="gt", bufs=4)

                for idx, j in enumerate(js):
                    ps = psum.tile([128, 1024], F32, tag="ps", name="ps")
                    for di, (dy, dx) in enumerate(DELTAS):
                        for cq in range(2):
                            r0 = 16 * cq
                            nc.tensor.matmul(
                                ps[:, 512 * cq:512 * (cq + 1)],
                                lhsT=wh_sb[0:NROW, 128 * di:128 * (di + 1)],
                                rhs=Rv[0:NROW, j, r0 + dy:r0 + dy + 16,
                                       dx:dx + 32],
                                start=(di == 0), stop=(di == 8))
                    span = slice(1024 * idx, 1024 * (idx + 1))
                    nc.scalar.activation(gates[0:96, span], ps[0:96, :],
                                         mybir.ActivationFunctionType.Sigmoid)
                    nc.scalar.activation(gates[96:128, span], ps[96:128, :],
                                         mybir.ActivationFunctionType.Tanh)
                    for G in range(4):
                        eng = dma_engines[dma_i % 2]
                        dma_i += 1
                        eng.dma_start(
                            out=gt[32 * idx:32 * idx + 32,
                                   1024 * G:1024 * (G + 1)],
                            in_=gates[32 * G:32 * G + 32, span])

                if nm == "B" and t > 1:
                    # previous step's exchange lands here: the sel matmuls
                    # sit between B's and A's matmuls in the tensor queue,
                    # and the vector halo copies run while B's pointwise
                    # still waits on its gates.
                    psel = psum.tile([64, 1024], F32, tag="psel",
                                     name="psel", bufs=1)
                    for cq in range(2):
                        cs = slice(512 * cq, 512 * (cq + 1))
                        nc.tensor.matmul(psel[0:32, cs], lhsT=slo[:, :],
                                         rhs=stg[:, 1024 + 512 * cq:
                                                 1536 + 512 * cq],
                                         start=True, stop=True)
                        nc.tensor.matmul(psel[32:64, cs], lhsT=shi[:, :],
                                         rhs=stg[:, cs],
                                         start=True, stop=True)
                    pse3 = psel[:, :].rearrange("p (y x) -> p y x",
                                                y=32, x=32)
                    # halos belong to the stack step t READS (written by
                    # exchange t-1)
                    nc.vector.tensor_copy(Rv[0:32, 0, 1:33, 1:33],
                                          pse3[0:32])
                    nc.vector.tensor_copy(Rv[64:96, 7, 1:33, 1:33],
                                          pse3[32:64])

                i_t = gt[0:PP, 0:1024]
                f_t = gt[0:PP, 1024:2048]
                o_t = gt[0:PP, 2048:3072]
                g_t = gt[0:PP, 3072:4096]
                c_sl = cst[0:PP, 1024 * blk:1024 * (blk + 1)]

                prod = work.tile([128, 1024], F32, tag="prod", name="prod")
                tmp = work.tile([128, 1024], F32, tag="tmp", name="tmp")
                tanhc = work.tile([128, 1024], F32, tag="tanhc", name="tanhc")
                h_t = work.tile([128, 1024], MM_DT, tag="ht", name="h_t",
                                bufs=3)
                nc.vector.tensor_mul(prod[0:PP, :], i_t, g_t)
                # slice A's f*c on gpsimd: shortens the exchange chain by
                # one DVE op
                if nm == "A":
                    nc.gpsimd.tensor_mul(tmp[0:PP, :], f_t, c_sl)
                else:
                    nc.vector.tensor_mul(tmp[0:PP, :], f_t, c_sl)
                nc.vector.tensor_add(c_sl, prod[0:PP, :], tmp[0:PP, :])
                nc.scalar.activation(tanhc[0:PP, :], c_sl,
                                     mybir.ActivationFunctionType.Tanh)
                nc.vector.tensor_mul(h_t[0:PP, :], o_t, tanhc[0:PP, :])

                ht3 = h_t[:, :].rearrange("p (y x) -> p y x", y=32, x=32)
                if t == T - 1:
                    for idx, j in enumerate(js):
                        nc.sync.dma_start(out=hout_d[:, j, :, :],
                                          in_=ht3[32 * idx:32 * idx + 32])
                else:
                    for idx, j in enumerate(js):
                        for g in range(3):
                            s = j + 1 - g
                            if 0 <= s < 8:
                                nc.vector.tensor_copy(
                                    Wv[32 * g:32 * g + 32, s, 1:33, 1:33],
                                    ht3[32 * idx:32 * idx + 32])
                    if nm == "A":
                        # boundary planes out -> collective -> readback.
                        # cin row c = [plane 8q | plane 8q+7]; gathered
                        # row 32r+c lands directly on staging partition
                        # 32r+c -> one contiguous readback DMA.
                        cin = dram.tile([32, 2048], MM_DT, tag="cin",
                                        name="cin")
                        gath = dram.tile([128, 2048], MM_DT, tag="gath",
                                         name="gath")
                        stg = work.tile([128, 2048], MM_DT, tag="stg",
                                        name="stg")
                        nc.sync.dma_start(out=cin[:, 0:1024],
                                          in_=h_t[0:32, :])
                        nc.sync.dma_start(out=cin[:, 1024:2048],
                                          in_=h_t[32:64, :])
                        nc.gpsimd.collective_compute(
                            "AllGather", mybir.AluOpType.bypass,
                            replica_groups=GROUPS,
                            ins=[cin[:, :].opt()],
                            outs=[gath[:, :].opt()])
                        nc.gpsimd.dma_start(out=stg[:, :], in_=gath[:, :])

    nc.finalize()
    _prog_cache["nc"] = nc
    return nc


def _host_inputs(input_batch, Wx, Wh, b):
    import ml_dtypes
    bf16 = ml_dtypes.bfloat16
    input_batch = np.asarray(input_batch, dtype=np.float32)
    Wx = np.asarray(Wx, dtype=np.float32)
    Wh = np.asarray(Wh, dtype=np.float32)
    b = np.asarray(b, dtype=np.float32)
    B = input_batch.shape[0]

    xp = np.zeros((B, T, 66, 66, 66), np.float32)
    xp[:, :, 1:65, 1:65, 1:65] = input_batch[:, :, 0]

    whl = np.zeros((9, NROW, 128), np.float32)
    for di, (dy, dx) in enumerate(DELTAS):
        for g in range(3):
            whl[di, 32 * g:32 * g + 32, :] = Wh[:, :, g, dy, dx].T
    whl[0, 96:123, :] = Wx[:, 0].reshape(128, 27).T
    whl[0, 123, :] = b

    # full-volume stride-2 im2col of x for every (b, t): [27, 32, 32, 32]
    xcol = np.zeros((B, T, 27, 32, 32, 32), np.float32)
    for bi in range(B):
        for t in range(T):
            for tz in range(3):
                for ty in range(3):
                    for tx in range(3):
                        tap = tz * 9 + ty * 3 + tx
                        xcol[bi, t, tap] = xp[bi, t, tz:tz + 64:2,
                                              ty:ty + 64:2, tx:tx + 64:2]

    # step 0 on host: gates from x-conv only (h_0 prev = 0, c_0 prev = 0)
    sig = lambda v: 0.5 * (1.0 + np.tanh(0.5 * v))
    wx27 = Wx[:, 0].reshape(128, 27)
    h0 = np.zeros((B, 32, 32, 32, 32), np.float32)
    c0 = np.zeros((B, 32, 32, 32, 32), np.float32)
    for bi in range(B):
        g0 = np.einsum('rzyx,pr->pzyx', xcol[bi, 0], wx27,
                       optimize=True) + b[:, None, None, None]
        c0[bi] = sig(g0[0:32]) * np.tanh(g0[96:128])
        h0[bi] = sig(g0[64:96]) * np.tanh(c0[bi])

    in_maps = []
    for c in range(8):
        bidx, q = divmod(c, 4)
        z0 = 8 * q

        stk0 = np.zeros((NROW, 8, PLW, PLW), np.float32)
        for s in range(8):
            for g in range(3):
                z = z0 + s - 1 + g
                if 0 <= z < 32:
                    stk0[32 * g:32 * g + 32, s, 1:33, 1:33] = h0[bidx][:, z]
            # xim for t=1 rides the initial stack
            stk0[96:123, s, 0:32, 0:32] = xcol[bidx, 1, :, z0 + s]
            stk0[123, s, 0:32, 0:32] = 1.0

        xim = np.zeros((8, 28, 8, PLW, PLW), np.float32)
        for ti, t in enumerate(range(2, T)):
            for s in range(8):
                xim[ti, 0:27, s, 0:32, 0:32] = xcol[bidx, t, :, z0 + s]
                xim[ti, 27, s, 0:32, 0:32] = 1.0

        c0c = np.zeros((128, 3, 1024), np.float32)
        for blk, js in ((0, [0, 7]), (1, [1, 2, 3, 4]), (2, [5, 6])):
            for idx, j in enumerate(js):
                c0c[32 * idx:32 * idx + 32, blk] = \
                    c0[bidx][:, z0 + j].reshape(32, 1024)

        selLo = np.zeros((128, 32), np.float32)
        selHi = np.zeros((128, 32), np.float32)
        if q > 0:
            selLo[32 * (q - 1) + np.arange(32), np.arange(32)] = 1.0
        if q < 3:
            selHi[32 * (q + 1) + np.arange(32), np.arange(32)] = 1.0

        in_maps.append({
            "stk0": stk0.reshape(NROW, 8 * PL).astype(bf16),
            "zer": np.zeros((96, 8 * PL), bf16),
            "xim": xim.reshape(8, 28, 8, PL).astype(bf16),
            "whl": whl.astype(bf16),
            "c0": c0c.reshape(128, 3 * 1024),
            "selLo": selLo.astype(bf16),
            "selHi": selHi.astype(bf16),
        })
    return in_maps


def run_cores(in_maps, **kwargs):
    nc = _build_program()
    return run_bass_kernel_spmd(nc, in_maps, list(range(8)), **kwargs)


def kernel(input_batch, Wx, Wh, b):
    in_maps = _host_inputs(input_batch, Wx, Wh, b)
    res = run_cores(in_maps)
    out = np.zeros((2, CH, 32, 32, 32), np.float32)
    for c in range(8):
        bidx, q = divmod(c, 4)
        out[bidx, :, 8 * q:8 * q + 8] = np.asarray(
            res.results[c]["hout"], dtype=np.float32)
    return out


# revision 17
# speedup vs baseline: 1.1154x; 1.1154x over previous
"""ConvLSTM3D encoder for 8 trn2 NeuronCores — halo-exchange version.

Sharding: core c handles batch c//4, z-quarter q = c%4 (output planes
[8q, 8q+8)).  Each core computes exactly its 8 planes every step (72
plane-steps/core vs the collective-free baseline's 144) and exchanges
the two boundary h planes with its z-neighbors after every step via a
4-core AllGather (groups {0..3}, {4..7}).

SPMD uniformity: the gathered buffer holds all 4 ranks' (bottom, top)
planes plus a zeroed pad block; each core pulls ITS two halo planes
straight into the h-stack with two register-offset DMAs whose row
indices are per-core input data (edge quarters point at the zero pad,
reproducing 'same' conv padding).  No matmul/PSUM hop on the halo path.

h-stack: double-buffered [124, 8 slots, 34, 34] bf16.  Partitions
0..95 hold 3 z-shift groups x 32 ch: group g at slot s holds plane
8q+s-1+g of the previous step's h, zero-padded borders.  Partitions
96..123 carry the per-step x-im2col taps (27) + ones row; they ride
pass 0 of the 9 (dy,dx) matmul passes, which also adds Wx-conv + bias.

Per plane: 9 matmul passes x 2 half-planes -> PSUM -> sigmoid/tanh ->
gate-major 'gates' -> 4 partition-crossing DMAs -> plane-major 'gt' ->
LSTM pointwise on DVE -> h_t copied into the write-stack (<=3 shift
slots).  c state never moves (fixed plane ownership).

Step slice order [B interior (j 4,3,2,1), C interior, A boundary]:
the boundary slice runs LAST so the collective it feeds has from +42
of step t until +25 of step t+1 to land — the tile scheduler's
conservative collective model and the real ~13us latency both fit.
"""

import os
import sys
from contextlib import ExitStack

import numpy as np

for _p in ("/opt/trn_rl_repo", "/root/.axon_site/_ro/trn_rl_repo"):
    if os.path.isdir(_p) and _p not in sys.path:
        sys.path.insert(0, _p)

import concourse.bass as bass
import concourse.bacc as bacc
import concourse.mybir as mybir
from concourse import tile
from concourse.bass_utils import run_bass_kernel_spmd

F32 = mybir.dt.float32
U32 = mybir.dt.uint32
MM_DT = mybir.dt.bfloat16
GT_DT = mybir.dt.bfloat16

T = 10             # total steps; step 0 on host, device runs t=1..9
CH = 32            # hidden channels
PLW = 34           # padded plane width
PL = PLW * PLW     # padded plane elements (1156)
NROW = 124         # contraction rows: 96 h + 27 x-taps + ones
DELTAS = [(dy, dx) for dy in range(3) for dx in range(3)]
# issue order: interior B (reversed so plane 1 is computed last, giving
# the previous step's boundary-plane copies time to land), interior C,
# boundary A last (feeds the exchange; consumed a full step later)
SLICES = [("B", [4, 3, 2, 1], 1), ("C", [5, 6], 2), ("A", [0, 7], 0)]
GROUPS = [[0, 1, 2, 3], [4, 5, 6, 7]]

_prog_cache = {}


def _build_program():
    if "nc" in _prog_cache:
        return _prog_cache["nc"]

    nc = bacc.Bacc(num_devices=8)

    stk0_d = nc.dram_tensor("stk0", [NROW, 8 * PL], MM_DT, kind="ExternalInput")
    zer_d = nc.dram_tensor("zer", [96, 8 * PL], MM_DT, kind="ExternalInput")
    xim_d = nc.dram_tensor("xim", [8, 28, 8, PL], MM_DT, kind="ExternalInput")
    whl_d = nc.dram_tensor("whl", [9, NROW, 128], MM_DT, kind="ExternalInput")
    c0_d = nc.dram_tensor("c0", [128, 3 * 1024], F32, kind="ExternalInput")
    hofs_d = nc.dram_tensor("hofs", [1, 2], U32, kind="ExternalInput")
    hout_d = nc.dram_tensor("hout", [CH, 8, 32, 32], MM_DT,
                            kind="ExternalOutput")

    with ExitStack() as ctx:
        tc = ctx.enter_context(tile.TileContext(nc))
        pers = ctx.enter_context(tc.tile_pool(name="pers", bufs=1))
        psum = ctx.enter_context(tc.tile_pool(name="psum", bufs=4, space="PSUM"))
        work = ctx.enter_context(tc.tile_pool(name="work", bufs=2))
        dram = ctx.enter_context(tc.tile_pool(name="dram", bufs=2, space="DRAM"))

        stkA = pers.tile([128, 8 * PL], MM_DT, tag="stkA", name="stkA")
        stkB = pers.tile([128, 8 * PL], MM_DT, tag="stkB", name="stkB")
        wh_sb = pers.tile([128, 9 * 128], MM_DT, tag="wh", name="wh_sb")
        cst = pers.tile([128, 3 * 1024], F32, tag="cst", name="cst")
        hofs = pers.tile([1, 2], U32, tag="hofs", name="hofs")

        ld_engines = [nc.sync, nc.scalar, nc.gpsimd]
        for d in range(9):
            ld_engines[d % 3].dma_start(
                out=wh_sb[0:NROW, 128 * d:128 * (d + 1)], in_=whl_d[d])
        nc.sync.dma_start(out=hofs[:, :], in_=hofs_d[:, :])

        # initial stack A (h_0 3-group + xim[t=1] + ones), B zeroed h-part
        quarter = 8 * PL // 4
        for qq in range(4):
            lo, hi = qq * quarter, (qq + 1) * quarter
            ld_engines[qq % 3].dma_start(out=stkA[0:NROW, lo:hi],
                                         in_=stk0_d[:, lo:hi])
            ld_engines[(qq + 1) % 3].dma_start(out=stkB[0:96, lo:hi],
                                               in_=zer_d[:, lo:hi])
        for qq in range(3):
            ld_engines[qq].dma_start(out=cst[:, 1024 * qq:1024 * (qq + 1)],
                                     in_=c0_d[:, 1024 * qq:1024 * (qq + 1)])

        # per-core gathered-row offsets for the two halo planes
        r_lo = nc.gpsimd.alloc_register("halo_lo")
        r_hi = nc.gpsimd.alloc_register("halo_hi")
        nc.gpsimd.reg_load(r_lo, hofs[0:1, 0:1])
        nc.gpsimd.reg_load(r_hi, hofs[0:1, 1:2])
        v_lo = nc.gpsimd.snap(r_lo, donate=True, min_val=0, max_val=128)
        v_hi = nc.gpsimd.snap(r_hi, donate=True, min_val=0, max_val=128)

        stacks = [stkA, stkB]
        dma_engines = [nc.sync, nc.scalar]
        dma_i = 0

        # one persistent exchange bounce pair (DRAM pool tiles are
        # bump-allocated, so allocate once and reuse; WAR across steps is
        # tracked).  gath rows 128:160 stay zero for the edge quarters.
        cin = dram.tile([32, 2048], MM_DT, tag="cin", name="cin")
        gath = dram.tile([160, 2048], MM_DT, tag="gath", name="gath")
        nc.scalar.dma_start(out=gath[128:160, :], in_=zer_d[0:32, 0:2048])
        gv = gath[:, :].rearrange("r (h y x) -> r h y x", h=2, y=32, x=32)

        for t in range(1, T):
            R = stacks[(t + 1) % 2]          # read stack (A at t=1)
            W = stacks[t % 2]                # write stack
            Rv = R[:, :].rearrange("p (s y x) -> p s y x", s=8, y=PLW, x=PLW)
            Wv = W[:, :].rearrange("p (s y x) -> p s y x", s=8, y=PLW, x=PLW)
            Wv2 = W[:, :].rearrange("p (s f) -> p s f", s=8, f=PL)

            # prefetch next step's x-im2col into the write stack's x rows
            if t < T - 1:
                nc.gpsimd.dma_start(out=Wv2[96:124, :, :], in_=xim_d[t - 1])

            for nm, js, blk in SLICES:
                npl = len(js)
                PP = 32 * npl
                gates = work.tile([128, 4096], GT_DT, tag="gates", name="gates")
                gt = work.tile([128, 4096], GT_DT, tag="gt", name="gt", bufs=4)

                for idx, j in enumerate(js):
                    ps = psum.tile([128, 1024], F32, tag="ps", name="ps")
                    for di, (dy, dx) in enumerate(DELTAS):
                        for cq in range(2):
                            r0 = 16 * cq
                            nc.tensor.matmul(
                                ps[:, 512 * cq:512 * (cq + 1)],
                                lhsT=wh_sb[0:NROW, 128 * di:128 * (di + 1)],
                                rhs=Rv[0:NROW, j, r0 + dy:r0 + dy + 16,
                                       dx:dx + 32],
                                start=(di == 0), stop=(di == 8))
                    span = slice(1024 * idx, 1024 * (idx + 1))
                    nc.scalar.activation(gates[0:96, span], ps[0:96, :],
                                         mybir.ActivationFunctionType.Sigmoid)
                    nc.scalar.activation(gates[96:128, span], ps[96:128, :],
                                         mybir.ActivationFunctionType.Tanh)
                    for G in range(4):
                        eng = dma_engines[dma_i % 2]
                        dma_i += 1
                        eng.dma_start(
                            out=gt[32 * idx:32 * idx + 32,
                                   1024 * G:1024 * (G + 1)],
                            in_=gates[32 * G:32 * G + 32, span])

                i_t = gt[0:PP, 0:1024]
                f_t = gt[0:PP, 1024:2048]
                o_t = gt[0:PP, 2048:3072]
                g_t = gt[0:PP, 3072:4096]
                c_sl = cst[0:PP, 1024 * blk:1024 * (blk + 1)]

                prod = work.tile([128, 1024], F32, tag="prod", name="prod")
                tmp = work.tile([128, 1024], F32, tag="tmp", name="tmp")
                tanhc = work.tile([128, 1024], F32, tag="tanhc", name="tanhc")
                h_t = work.tile([128, 1024], MM_DT, tag="ht", name="h_t",
                                bufs=3)
                nc.vector.tensor_mul(prod[0:PP, :], i_t, g_t)
                nc.vector.tensor_mul(tmp[0:PP, :], f_t, c_sl)
                nc.vector.tensor_add(c_sl, prod[0:PP, :], tmp[0:PP, :])
                nc.scalar.activation(tanhc[0:PP, :], c_sl,
                                     mybir.ActivationFunctionType.Tanh)
                nc.vector.tensor_mul(h_t[0:PP, :], o_t, tanhc[0:PP, :])

                ht3 = h_t[:, :].rearrange("p (y x) -> p y x", y=32, x=32)
                if t == T - 1:
                    for idx, j in enumerate(js):
                        nc.sync.dma_start(out=hout_d[:, j, :, :],
                                          in_=ht3[32 * idx:32 * idx + 32])
                else:
                    for idx, j in enumerate(js):
                        for g in range(3):
                            s = j + 1 - g
                            if 0 <= s < 8:
                                nc.vector.tensor_copy(
                                    Wv[32 * g:32 * g + 32, s, 1:33, 1:33],
                                    ht3[32 * idx:32 * idx + 32])
                    if nm == "A":
                        # boundary planes out -> AllGather -> two
                        # register-offset DMAs pull this core's halo
                        # planes straight into the write stack.  cin row
                        # c = [plane 8q | plane 8q+7]; gathered row 32r+c
                        # = rank r's cin row c.
                        nc.sync.dma_start(out=cin[:, 0:1024],
                                          in_=h_t[0:32, :])
                        nc.sync.dma_start(out=cin[:, 1024:2048],
                                          in_=h_t[32:64, :])
                        nc.gpsimd.collective_compute(
                            "AllGather", mybir.AluOpType.bypass,
                            replica_groups=GROUPS,
                            ins=[cin[:, :].opt()],
                            outs=[gath[0:128, :].opt()])
                        nc.gpsimd.dma_start(
                            out=Wv[0:32, 0, 1:33, 1:33],
                            in_=gv[bass.ds(v_lo, 32), 1])
                        nc.gpsimd.dma_start(
                            out=Wv[64:96, 7, 1:33, 1:33],
                            in_=gv[bass.ds(v_hi, 32), 0])

    nc.finalize()
    _prog_cache["nc"] = nc
    return nc


def _host_inputs(input_batch, Wx, Wh, b):
    import ml_dtypes
    bf16 = ml_dtypes.bfloat16
    input_batch = np.asarray(input_batch, dtype=np.float32)
    Wx = np.asarray(Wx, dtype=np.float32)
    Wh = np.asarray(Wh, dtype=np.float32)
    b = np.asarray(b, dtype=np.float32)
    B = input_batch.shape[0]

    xp = np.zeros((B, T, 66, 66, 66), np.float32)
    xp[:, :, 1:65, 1:65, 1:65] = input_batch[:, :, 0]

    whl = np.zeros((9, NROW, 128), np.float32)
    for di, (dy, dx) in enumerate(DELTAS):
        for g in range(3):
            whl[di, 32 * g:32 * g + 32, :] = Wh[:, :, g, dy, dx].T
    whl[0, 96:123, :] = Wx[:, 0].reshape(128, 27).T
    whl[0, 123, :] = b

    # full-volume stride-2 im2col of x for every (b, t): [27, 32, 32, 32]
    xcol = np.zeros((B, T, 27, 32, 32, 32), np.float32)
    for bi in range(B):
        for t in range(T):
            for tz in range(3):
                for ty in range(3):
                    for tx in range(3):
                        tap = tz * 9 + ty * 3 + tx
                        xcol[bi, t, tap] = xp[bi, t, tz:tz + 64:2,
                                              ty:ty + 64:2, tx:tx + 64:2]

    # step 0 on host: gates from x-conv only (h_0 prev = 0, c_0 prev = 0)
    sig = lambda v: 0.5 * (1.0 + np.tanh(0.5 * v))
    wx27 = Wx[:, 0].reshape(128, 27)
    h0 = np.zeros((B, 32, 32, 32, 32), np.float32)
    c0 = np.zeros((B, 32, 32, 32, 32), np.float32)
    for bi in range(B):
        g0 = np.einsum('rzyx,pr->pzyx', xcol[bi, 0], wx27,
                       optimize=True) + b[:, None, None, None]
        c0[bi] = sig(g0[0:32]) * np.tanh(g0[96:128])
        h0[bi] = sig(g0[64:96]) * np.tanh(c0[bi])

    in_maps = []
    for c in range(8):
        bidx, q = divmod(c, 4)
        z0 = 8 * q

        stk0 = np.zeros((NROW, 8, PLW, PLW), np.float32)
        for s in range(8):
            for g in range(3):
                z = z0 + s - 1 + g
                if 0 <= z < 32:
                    stk0[32 * g:32 * g + 32, s, 1:33, 1:33] = h0[bidx][:, z]
            # xim for t=1 rides the initial stack
            stk0[96:123, s, 0:32, 0:32] = xcol[bidx, 1, :, z0 + s]
            stk0[123, s, 0:32, 0:32] = 1.0

        xim = np.zeros((8, 28, 8, PLW, PLW), np.float32)
        for ti, t in enumerate(range(2, T)):
            for s in range(8):
                xim[ti, 0:27, s, 0:32, 0:32] = xcol[bidx, t, :, z0 + s]
                xim[ti, 27, s, 0:32, 0:32] = 1.0

        c0c = np.zeros((128, 3, 1024), np.float32)
        for nm, js, blk in SLICES:
            for idx, j in enumerate(js):
                c0c[32 * idx:32 * idx + 32, blk] = \
                    c0[bidx][:, z0 + j].reshape(32, 1024)

        hofs = np.array([[32 * (q - 1) if q > 0 else 128,
                          32 * (q + 1)]], np.uint32)

        in_maps.append({
            "stk0": stk0.reshape(NROW, 8 * PL).astype(bf16),
            "zer": np.zeros((96, 8 * PL), bf16),
            "xim": xim.reshape(8, 28, 8, PL).astype(bf16),
            "whl": whl.astype(bf16),
            "c0": c0c.reshape(128, 3 * 1024),
            "hofs": hofs,
        })
    return in_maps


def run_cores(in_maps, **kwargs):
    nc = _build_program()
    return run_bass_kernel_spmd(nc, in_maps, list(range(8)), **kwargs)


def kernel(input_batch, Wx, Wh, b):
    in_maps = _host_inputs(input_batch, Wx, Wh, b)
    res = run_cores(in_maps)
    out = np.zeros((2, CH, 32, 32, 32), np.float32)
    for c in range(8):
        bidx, q = divmod(c, 4)
        out[bidx, :, 8 * q:8 * q + 8] = np.asarray(
            res.results[c]["hout"], dtype=np.float32)
    return out
